# revision 14
# baseline (speedup 1.0000x reference)
"""Multi-head attention block for Trainium2, SPMD over 8 NeuronCores. v4.

Sharding: 8 shards = batch (4) x head-group (2 groups of 6 heads).
Per core (b, g), for its 6 heads:
    qkv   = x[b] @ Wqkv[:, cols(g)]            (bf16 matmul, fp32 accum)
    S^T_h = K_h Q_h^T   per head               (keys on partitions)
      - heads processed in PAIRS (2p, 2p+1) at PE row groups 0-63 / 64-127
        so consecutive score matmuls run concurrently.
    P^T_h = exp(SCALE * S^T_h) -> bf16         (ACT table exp / DVE
        Schraudolph int16 bit-trick exp, split across both engines)
    PV (v4, transposed):  outT_h[d, q] = [V_h | 1]^T @ P^T_h
      - stationary = [V_h | ones] (65 cols, LDW hidden), streaming = P^T
        tiles at N=512 -> no per-MM LDWEIGHTS bottleneck, and the output
        lands directly in the ogT layout needed by proj (no transposes).
      - row 64 of the PV psum = softmax denominator for 512 queries.
    den chain: psum -> sbuf copy -> DRAM bounce -> compact [128,4] recip
        (DVE recip is 8 cyc/elem/lane; compact keeps it ~200ns) -> DRAM ->
        broadcast-read to [64, 512] -> tensor_tensor normalize into ogT.
    y_g   = ogT^T @ Wproj[rows(g), :]          (partial, fp32 out)
Host sums the two head-group partials per batch and adds bproj.

Shapes hardcoded: x [4, 2048, 768], Wqkv [768, 2304], Wproj [768, 768].
"""

import os
from contextlib import ExitStack

import numpy as np
import ml_dtypes

import concourse.bass as bass
import concourse.mybir as mybir
import concourse.tile as tile
from concourse import bacc
from concourse.bass_utils import run_bass_kernel_spmd
from concourse.masks import make_identity

B, N, C = 4, 2048, 768
H, D = 12, 64          # total heads, head dim
G = 2                  # head groups (tensor-parallel axis)
HL = H // G            # heads per core = 6
SCALE = D ** -0.5
P = 128
CB = C // P            # 6 contraction blocks
NT = N // P            # 16 row tiles
NHALF = N // 1024      # 2 query halves
EG = HL * D            # 384 = per-group width of Q / K / V
NCORES = 8

f32 = mybir.dt.float32
bf16 = mybir.dt.bfloat16
i16 = mybir.dt.int16

# Schraudolph bf16 exp: exp(x) ~= bitcast_bf16(int16(x * 128/ln2 + B))
EXP_A = float(128.0 / np.log(2.0))
EXP_B = 16250.5

DVE_PER_8 = int(os.environ.get("KRN_DVE_PER_8", "3"))  # of 8 exp tiles -> DVE
PT_BUFS = int(os.environ.get("KRN_PT_BUFS", "55"))
PVC_BUFS = int(os.environ.get("KRN_PVC_BUFS", "3"))
TAIL_SPINS = int(os.environ.get("KRN_TAIL_SPINS", "30"))
WARM_MMS = int(os.environ.get("KRN_WARM", "64"))


def _build_program():
    nc = bacc.Bacc("TRN2", target_bir_lowering=False, debug=False)

    xT = nc.dram_tensor("xT", [C, N], bf16, kind="ExternalInput")           # x[b].T
    wqkv = nc.dram_tensor("wqkv", [C, 3 * EG], bf16, kind="ExternalInput")  # [Qg|Kg|Vg]
    wproj = nc.dram_tensor("wproj", [EG, C], bf16, kind="ExternalInput")    # group rows
    y = nc.dram_tensor("y", [N, C], bf16, kind="ExternalOutput")            # partial out
    # den/recip DRAM bounce scratch: one row per (head, half, qchunk)
    den_d = nc.dram_tensor("den_d", [HL * 2, 1024], bf16, kind="Internal")
    rcp_d = nc.dram_tensor("rcp_d", [HL * 2, 1024], bf16, kind="Internal")

    with tile.TileContext(nc) as tc, ExitStack() as ctx:
        persist = ctx.enter_context(tc.tile_pool(name="persist", bufs=1))
        ptpool = ctx.enter_context(tc.tile_pool(name="ptpool", bufs=PT_BUFS))
        ypool = ctx.enter_context(tc.tile_pool(name="ypool", bufs=3))
        pvcpool = ctx.enter_context(tc.tile_pool(name="pvc", bufs=PVC_BUFS))
        rcppool = ctx.enter_context(tc.tile_pool(name="rcp", bufs=3))
        nwtpool = ctx.enter_context(tc.tile_pool(name="nwt", bufs=6))
        dencpool = ctx.enter_context(tc.tile_pool(name="denc", bufs=4))
        ps_sc = ctx.enter_context(tc.tile_pool(name="ps_sc", bufs=3, space="PSUM"))
        ps_pv = ctx.enter_context(tc.tile_pool(name="ps_pv", bufs=2, space="PSUM"))

        identity = persist.tile([P, P], bf16, tag="identity")
        make_identity(nc, identity)

        # PE clock warm-up spin + ACT exp-table preload, overlapping the
        # input DMA window (identity is generated on-chip, no DMA deps).
        warm_sb = persist.tile([P, 16], bf16, tag="warm")
        nc.scalar.activation(warm_sb[:], identity[:, :16],
                             mybir.ActivationFunctionType.Exp)
        for w in range(WARM_MMS):
            wps = ps_sc.tile([P, P], f32, tag="sc", name=f"warm{w}")
            nc.tensor.matmul(wps[:], identity[:], identity[:],
                             start=True, stop=True)

        # ---- persistent tiles ----
        # Load order: K cols + x half-0 + Q cols first (feed first scores),
        # then x half-1, V, wproj.
        wq_sb = persist.tile([P, CB, 3 * EG], bf16, tag="wq")
        wqv = wqkv[:].rearrange("(cb p) e -> p cb e", p=P)
        nc.sync.dma_start(wq_sb[:, :, EG : 2 * EG], wqv[:, :, EG : 2 * EG])   # K
        xts = [[ptpool.tile([P, 1024], bf16, tag="pt", name=f"xt{cb}_{hf}")
                for hf in range(NHALF)] for cb in range(CB)]
        for cb in range(CB):
            nc.sync.dma_start(xts[cb][0][:], xT[cb * P : (cb + 1) * P, 0:1024])
        nc.sync.dma_start(wq_sb[:, :, 0:EG], wqv[:, :, 0:EG])                 # Q
        for cb in range(CB):
            nc.sync.dma_start(xts[cb][1][:], xT[cb * P : (cb + 1) * P, 1024:2048])
        nc.sync.dma_start(wq_sb[:, :, 2 * EG : 3 * EG], wqv[:, :, 2 * EG :])  # V
        wp_sb = persist.tile([P, EG // P, C], bf16, tag="wp")
        nc.sync.dma_start(wp_sb[:], wproj[:].rearrange("(cb p) c -> p cb c", p=P))

        qkT = persist.tile([P, 2 * EG // P, N], bf16, tag="qkT")   # Q blocks 0-2, K 3-5
        # V stationary: per (mt, head): [V_h(64) | ones(1)]
        vst = persist.tile([P, NT, HL, D + 1], bf16, tag="vst")
        nc.vector.memset(vst[:, :, :, D : D + 1], 1.0)
        pones = persist.tile([P, D], bf16, tag="pones")
        nc.vector.memset(pones[:], 1.0)
        ogTs = [persist.tile([P, N], bf16, tag=f"ogT{cb}", name=f"ogT{cb}")
                for cb in range(EG // P)]

        copy_rr = [0]

        def copy_any(dst, src):
            """Alternate psum->sbuf copies between DVE and ACT."""
            copy_rr[0] ^= 1
            if copy_rr[0]:
                nc.vector.tensor_copy(dst, src)
            else:
                nc.scalar.copy(dst, src)

        # ---- QKV units ----
        def qk_unit(eb, hf):
            """One [128,1024] chunk of (QKV)^T block eb, query-half hf."""
            def go():
                psum = ps_sc.tile([P, 1024], f32, tag="sc", name=f"qk{eb}_{hf}")
                for sub in range(2):
                    for cb in range(CB):
                        nc.tensor.matmul(
                            psum[:, sub * 512 : (sub + 1) * 512],
                            wq_sb[:, cb, eb * P : (eb + 1) * P],
                            xts[cb][hf][:, sub * 512 : (sub + 1) * 512],
                            start=(cb == 0),
                            stop=(cb == CB - 1),
                        )
                copy_any(qkT[:, eb, hf * 1024 : (hf + 1) * 1024], psum[:])
            return go

        def v_unit(mt):
            def go():
                vpsum = ps_pv.tile([P, 512], f32, tag="pv", name=f"v{mt}")
                for cb in range(CB):
                    nc.tensor.matmul(
                        vpsum[:, 0:EG],
                        xts[cb][mt // 8][:, (mt % 8) * P : (mt % 8 + 1) * P],
                        wq_sb[:, cb, 2 * EG : 3 * EG],
                        start=(cb == 0),
                        stop=(cb == CB - 1),
                    )
                copy_any(
                    vst[:, mt, :, 0:D],
                    vpsum[:, 0:EG].rearrange("p (h d) -> p h d", d=D),
                )
            return go

        # ---- scores + exp for one head-pair over one query half ----
        def emit_scores_pair(p, hf, work):
            """Per (mt, sub): ONE shared psum tile [128,1024] = [e|o]: head 2p
            (PE rows 0-63) -> cols 0:512, head 2p+1 (rows 64-127) -> cols
            512:1024. Shared tile readiness keeps the e/o matmuls adjacent in
            the schedule so they run concurrently (disjoint row groups).
            exp alternates ACT/DVE per tile; `work` closures are spread
            through the 16 mt steps."""
            kblk = 3 + p
            qblk = p
            pts = [[ptpool.tile([P, 1024], bf16, tag="pt", name=f"pt{p}{hf}_{mt}_{sub}")
                    for sub in range(2)] for mt in range(NT)]
            wi = 0
            for mt in range(NT):
                for sub in range(2):
                    ps = ps_sc.tile([P, 1024], f32, tag="sc", name=f"s{p}{hf}_{mt}_{sub}")
                    off = hf * 1024 + sub * 512
                    nc.tensor.matmul(
                        ps[:, 0:512],
                        qkT[0:D, kblk, mt * P : (mt + 1) * P],
                        qkT[0:D, qblk, off : off + 512], start=True, stop=True,
                    )
                    nc.tensor.matmul(
                        ps[:, 512:1024],
                        qkT[D:P, kblk, mt * P : (mt + 1) * P],
                        qkT[D:P, qblk, off : off + 512], start=True, stop=True,
                    )
                    # exp in two 512-halves on BOTH engines concurrently: the
                    # psum tile frees after max(ACT, DVE) instead of one
                    # serial 1024-wide op on a backlogged queue.
                    dve_lo = (mt * 2 + sub) % 2 == 0
                    for lo, hi, dve in ((0, 512, dve_lo), (512, 1024, not dve_lo)):
                        if dve:
                            nc.vector.tensor_scalar(
                                pts[mt][sub][:, lo:hi].bitcast(i16), ps[:, lo:hi],
                                EXP_A * SCALE, EXP_B,
                                mybir.AluOpType.mult, mybir.AluOpType.add,
                            )
                        else:
                            nc.scalar.activation(
                                pts[mt][sub][:, lo:hi], ps[:, lo:hi],
                                mybir.ActivationFunctionType.Exp, scale=SCALE,
                            )
                hi = (mt + 1) * len(work) // NT
                while wi < hi:
                    work[wi]()
                    wi += 1
            return pts

        # ---- PV (transposed) + den/normalize chain ----
        def pv_mm(h, hf, qc, pts, pvc):
            """16 accumulating MMs: outT_h rows 0-63 + den row 64, then copy
            psum -> sbuf staging (frees the psum bank fast). Queries qc*512..
            of this half live in pts[mt][qc], cols (h%2)*512..+512."""
            def go():
                ho = (h % 2) * 512
                pvp = ps_pv.tile([P, 512], f32, tag="pv", name=f"pv{h}{hf}{qc}")
                for mt in range(NT):
                    nc.tensor.matmul(
                        pvp[0 : D + 1, :],
                        vst[:, mt, h, :],
                        pts[mt][qc][:, ho : ho + 512],
                        start=(mt == 0),
                        stop=(mt == NT - 1),
                    )
                copy_any(pvc[0 : D + 1, qc * 512 : (qc + 1) * 512],
                         pvp[0 : D + 1, :])
            return go

        def den_norm(h, hf, qa, qb):
            """den cols [qa*512, qb*512) -> DRAM -> compact recip -> DRAM ->
            bcast [64, L] -> normalize into ogT. Batched (qa,qb)=(0,2) for
            steady state; per-qc for the tail pair."""
            def go():
                j = h * 2 + hf
                L = (qb - qa) * 512
                qs = slice(qa * 512, qb * 512)
                nc.gpsimd.dma_start(
                    den_d[j, qs].rearrange("(o q) -> o q", o=1), pvc_of[h][D : D + 1, qs]
                )
                denc = dencpool.tile([P, 8], bf16, tag="dc", name=f"dc{j}_{qa}")
                nc.gpsimd.dma_start(
                    denc[:, 0 : L // P], den_d[j, qs].rearrange("(p f) -> p f", p=P)
                )
                rcc = dencpool.tile([P, 8], bf16, tag="dc", name=f"rc{j}_{qa}")
                with nc.allow_low_precision(reason="bf16 softmax 1/den"):
                    nc.vector.reciprocal(rcc[:, 0 : L // P], denc[:, 0 : L // P])
                nc.gpsimd.dma_start(
                    rcp_d[j, qs].rearrange("(p f) -> p f", p=P), rcc[:, 0 : L // P]
                )
                rcpb = rcppool.tile([D, 1024], bf16, tag="rb", name=f"rb{j}_{qa}")
                nc.gpsimd.dma_start(
                    rcpb[:, 0:L],
                    rcp_d[j, qs].rearrange("q -> () q").broadcast_to([D, L]),
                )
                cb, half = h // 2, h % 2
                nc.vector.tensor_tensor(
                    ogTs[cb][half * D : (half + 1) * D,
                             hf * 1024 + qa * 512 : hf * 1024 + qb * 512],
                    pvc_of[h][0:D, qs], rcpb[:, 0:L], mybir.AluOpType.mult,
                )
            return go

        # bf16 magic-constant reciprocal + one Newton step (tail only):
        # den row -> PE broadcast [64,512] -> r0 = bitcast(0x7EF3 - bits(d))
        # -> r1 = r0*(2 - d*r0) -> normalize. ~2.5us latency, no DMA.
        RCP_MAGIC = float(0x7EF3)

        def den_newton(h, hf, qc):
            def go():
                qs = slice(qc * 512, (qc + 1) * 512)
                pvc = pvc_of[h]
                denb = ps_pv.tile([P, 512], f32, tag="pv", name=f"dnb{h}{qc}")
                nc.tensor.matmul(denb[0:D, :], pones[64:65, :], pvc[64:65, qs],
                                 start=True, stop=True)
                dbs = nwtpool.tile([D, 512], bf16, tag="nw", name=f"dbs{h}{qc}")
                r0 = nwtpool.tile([D, 512], bf16, tag="nw", name=f"r0{h}{qc}")
                tt = nwtpool.tile([D, 512], bf16, tag="nw", name=f"tt{h}{qc}")
                nc.scalar.copy(dbs[:], denb[0:D, :])
                nc.vector.tensor_scalar(
                    r0[:].bitcast(i16), dbs[:].bitcast(i16), -1.0, RCP_MAGIC,
                    mybir.AluOpType.mult, mybir.AluOpType.add,
                )
                nc.vector.tensor_tensor(tt[:], dbs[:], r0[:], mybir.AluOpType.mult)
                nc.vector.tensor_scalar(dbs[:], tt[:], -1.0, 2.0,
                                        mybir.AluOpType.mult, mybir.AluOpType.add)
                nc.vector.tensor_tensor(tt[:], r0[:], dbs[:], mybir.AluOpType.mult)
                cb, half = h // 2, h % 2
                nc.vector.tensor_tensor(
                    ogTs[cb][half * D : (half + 1) * D,
                             hf * 1024 + qc * 512 : hf * 1024 + (qc + 1) * 512],
                    pvc[0:D, qs], tt[:], mybir.AluOpType.mult,
                )
            return go

        pvc_of = {}

        def pv_pair(p, hf, pts, split_den=False):
            """Work closures for both heads of pair p on half hf."""
            units = []
            for h in (2 * p, 2 * p + 1):
                pvc_of[h] = pvcpool.tile([D + 1, 1024], bf16, tag="pvc",
                                         name=f"pvc{h}{hf}")
                pvc = pvc_of[h]
                if split_den:
                    for qc in range(2):
                        units.append(pv_mm(h, hf, qc, pts, pvc))
                        units.append(den_newton(h, hf, qc))
                else:
                    units.append(pv_mm(h, hf, 0, pts, pvc))
                    units.append(pv_mm(h, hf, 1, pts, pvc))
                    units.append(den_norm(h, hf, 0, 2))
            return units

        # ---- proj ----
        yv = y[:].rearrange("(nt p) c -> p nt c", p=P)

        def proj_unit(nt):
            def go():
                y_sb = ypool.tile([P, C], bf16, tag="y", name=f"y{nt}")
                for halfc in range(2):
                    ppsum = ps_pv.tile([P, 512], f32, tag="pv", name=f"pj{nt}_{halfc}")
                    for cb in range(EG // P):
                        nc.tensor.matmul(
                            ppsum[:, 0:EG],
                            ogTs[cb][:, nt * P : (nt + 1) * P],
                            wp_sb[:, cb, halfc * EG : (halfc + 1) * EG],
                            start=(cb == 0),
                            stop=(cb == EG // P - 1),
                        )
                    copy_any(y_sb[:, halfc * EG : (halfc + 1) * EG], ppsum[:, 0:EG])
                nc.sync.dma_start(yv[:, nt, :], y_sb[:])
            return go

        # ---- emission schedule ----
        # Phases hf-major: (p, hf) = 00,10,20,01,11,21. Pair p's scores need
        # K block (3+p) over both key halves and Q block p for half hf.
        # PV of phase i rides phase i+1's score emission.
        qk_unit(3, 0)()
        qk_unit(3, 1)()
        qk_unit(0, 0)()
        pending = {
            0: [qk_unit(4, 0), qk_unit(4, 1), qk_unit(1, 0)]
               + [v_unit(m) for m in range(NT)],
            1: [qk_unit(5, 0), qk_unit(5, 1), qk_unit(2, 0)],
            2: [qk_unit(0, 1)],
            3: [qk_unit(1, 1)],
            4: [qk_unit(2, 1)] + [proj_unit(nt) for nt in range(6)],
            5: [proj_unit(6), proj_unit(7)],
        }
        order = [(0, 0), (1, 0), (2, 0), (0, 1), (1, 1), (2, 1)]
        prev_pv = []
        for i, (p, hf) in enumerate(order):
            work = list(prev_pv) + pending.get(i, [])
            pts = emit_scores_pair(p, hf, work)
            prev_pv = pv_pair(p, hf, pts, split_den=(i == 5))
        # tail: last pair's PV qc0 bursts + chains first, then qc1, then
        # warm-keeper spins (bridge the den-chain latency without letting
        # the PE HAM re-throttle), then proj of half 1.
        e0, de0, e1, de1, o0, do0, o1, do1 = prev_pv
        e0(); de0(); o0(); do0()
        e1(); de1(); o1(); do1()
        spin_ps = ps_sc.tile([P, 64], f32, tag="sc", name="tailspin")
        for w in range(TAIL_SPINS):
            nc.tensor.matmul(spin_ps[:, 0:32], identity[:], identity[:, 0:32],
                             start=True, stop=True)
        for nt in range(8, NT):
            proj_unit(nt)()

    nc.compile()
    return nc


_PROGRAM = None


def _get_program():
    global _PROGRAM
    if _PROGRAM is None:
        _PROGRAM = _build_program()
    return _PROGRAM


def _shard_inputs(x, Wqkv, Wproj):
    bf = ml_dtypes.bfloat16
    in_maps = []
    for core in range(NCORES):
        b, g = core // G, core % G
        xT = np.ascontiguousarray(x[b].T).astype(bf)
        wg = np.concatenate(
            [
                Wqkv[:, g * EG : (g + 1) * EG],
                Wqkv[:, C + g * EG : C + (g + 1) * EG],
                Wqkv[:, 2 * C + g * EG : 2 * C + (g + 1) * EG],
            ],
            axis=1,
        ).astype(bf)
        wp = np.ascontiguousarray(Wproj[g * EG : (g + 1) * EG, :]).astype(bf)
        in_maps.append({"xT": xT, "wqkv": wg, "wproj": wp})
    return in_maps


def _run(x, Wqkv, Wproj, bproj, trace=False):
    nc = _get_program()
    in_maps = _shard_inputs(x, Wqkv, Wproj)
    res = run_bass_kernel_spmd(nc, in_maps, list(range(NCORES)), trace=trace)
    out = np.empty((B, N, C), np.float32)
    for b in range(B):
        out[b] = (
            res.results[b * G]["y"].astype(np.float32)
            + res.results[b * G + 1]["y"].astype(np.float32)
            + bproj
        )
    return out, res


def kernel(x, Wqkv, Wproj, bproj):
    x = np.asarray(x, np.float32)
    Wqkv = np.asarray(Wqkv, np.float32)
    Wproj = np.asarray(Wproj, np.float32)
    bproj = np.asarray(bproj, np.float32)
    out, _ = _run(x, Wqkv, Wproj, bproj)
    return out


# revision 17
# speedup vs baseline: 1.1709x; 1.1709x over previous
"""Multi-head attention block for Trainium2, SPMD over 8 NeuronCores. v4.

Sharding: 8 shards = batch (4) x head-group (2 groups of 6 heads).
Per core (b, g), for its 6 heads:
    qkv   = x[b] @ Wqkv[:, cols(g)]            (bf16 matmul, fp32 accum)
    S^T_h = K_h Q_h^T   per head               (keys on partitions)
      - heads processed in PAIRS (2p, 2p+1) at PE row groups 0-63 / 64-127
        so consecutive score matmuls run concurrently.
    P^T_h = exp(SCALE * S^T_h) -> bf16         (ACT table exp / DVE
        Schraudolph int16 bit-trick exp, split across both engines)
    PV (v4, transposed):  outT_h[d, q] = [V_h | 1]^T @ P^T_h
      - stationary = [V_h | ones] (65 cols, LDW hidden), streaming = P^T
        tiles at N=512 -> no per-MM LDWEIGHTS bottleneck, and the output
        lands directly in the ogT layout needed by proj (no transposes).
      - row 64 of the PV psum = softmax denominator for 512 queries.
    den chain: psum -> sbuf copy -> DRAM bounce -> compact [128,4] recip
        (DVE recip is 8 cyc/elem/lane; compact keeps it ~200ns) -> DRAM ->
        broadcast-read to [64, 512] -> tensor_tensor normalize into ogT.
    y_g   = ogT^T @ Wproj[rows(g), :]          (partial, fp32 out)
Host sums the two head-group partials per batch and adds bproj.

Shapes hardcoded: x [4, 2048, 768], Wqkv [768, 2304], Wproj [768, 768].
"""

import os
from contextlib import ExitStack

import numpy as np
import ml_dtypes

import concourse.bass as bass
import concourse.mybir as mybir
import concourse.tile as tile
from concourse import bacc
from concourse.bass_utils import run_bass_kernel_spmd

B, N, C = 4, 2048, 768
H, D = 12, 64          # total heads, head dim
G = 2                  # head groups (tensor-parallel axis)
HL = H // G            # heads per core = 6
SCALE = D ** -0.5
P = 128
CB = C // P            # 6 contraction blocks
NT = N // P            # 16 row tiles
NHALF = N // 1024      # 2 query halves
EG = HL * D            # 384 = per-group width of Q / K / V
NCORES = 8

f32 = mybir.dt.float32
bf16 = mybir.dt.bfloat16
i16 = mybir.dt.int16

# Schraudolph bf16 exp: exp(x) ~= bitcast_bf16(int16(x * 128/ln2 + B))
EXP_A = float(128.0 / np.log(2.0))
EXP_B = 16250.5

DVE_PER_8 = int(os.environ.get("KRN_DVE_PER_8", "4"))  # of 8 exp tiles -> DVE
PT_BUFS = int(os.environ.get("KRN_PT_BUFS", "55"))
PVC_BUFS = int(os.environ.get("KRN_PVC_BUFS", "3"))
TAIL_SPINS = int(os.environ.get("KRN_TAIL_SPINS", "30"))
WARM_MMS = int(os.environ.get("KRN_WARM", "64"))


def _build_program():
    nc = bacc.Bacc("TRN2", target_bir_lowering=False, debug=False)

    xT = nc.dram_tensor("xT", [C, N], bf16, kind="ExternalInput")           # x[b].T
    wqkv = nc.dram_tensor("wqkv", [C, 3 * EG], bf16, kind="ExternalInput")  # [Qg|Kg|Vg]
    wproj = nc.dram_tensor("wproj", [EG, C], bf16, kind="ExternalInput")    # group rows
    y = nc.dram_tensor("y", [N, C], bf16, kind="ExternalOutput")            # partial out
    # den/recip DRAM bounce scratch: one row per (head, half, qchunk)
    den_d = nc.dram_tensor("den_d", [HL * 2, 1024], bf16, kind="Internal")
    rcp_d = nc.dram_tensor("rcp_d", [HL * 2, 1024], bf16, kind="Internal")

    with tile.TileContext(nc) as tc, ExitStack() as ctx:
        persist = ctx.enter_context(tc.tile_pool(name="persist", bufs=1))
        ptpool = ctx.enter_context(tc.tile_pool(name="ptpool", bufs=PT_BUFS))
        ypool = ctx.enter_context(tc.tile_pool(name="ypool", bufs=3))
        pvcpool = ctx.enter_context(tc.tile_pool(name="pvc", bufs=PVC_BUFS))
        rcppool = ctx.enter_context(tc.tile_pool(name="rcp", bufs=3))
        nwtpool = ctx.enter_context(tc.tile_pool(name="nwt", bufs=6))
        dencpool = ctx.enter_context(tc.tile_pool(name="denc", bufs=4))
        ps_sc = ctx.enter_context(tc.tile_pool(name="ps_sc", bufs=3, space="PSUM"))
        ps_pv = ctx.enter_context(tc.tile_pool(name="ps_pv", bufs=2, space="PSUM"))

        # PE clock warm-up spin + ACT exp-table preload, overlapping the
        # input DMA window. Spins run on an uninitialized tile (values are
        # irrelevant, psum is discarded) so nothing gates the first matmul.
        junk = persist.tile([P, P], bf16, tag="junk")
        nc.vector.memset(junk[:], 1.0)
        warm_sb = persist.tile([P, 16], bf16, tag="warm")
        nc.scalar.activation(warm_sb[:], junk[:, :16],
                             mybir.ActivationFunctionType.Exp)
        for w in range(WARM_MMS):
            wps = ps_sc.tile([P, P], f32, tag="sc", name=f"warm{w}")
            nc.tensor.matmul(wps[:], junk[:], junk[:],
                             start=True, stop=True)

        # ---- persistent tiles ----
        # Load order: K cols + x half-0 + Q cols first (feed first scores),
        # then x half-1, V, wproj.
        wq_sb = persist.tile([P, CB, 3 * EG], bf16, tag="wq")
        wqv = wqkv[:].rearrange("(cb p) e -> p cb e", p=P)
        nc.sync.dma_start(wq_sb[:, :, EG : 2 * EG], wqv[:, :, EG : 2 * EG])   # K
        xts = [[ptpool.tile([P, 1024], bf16, tag="pt", name=f"xt{cb}_{hf}")
                for hf in range(NHALF)] for cb in range(CB)]
        for cb in range(CB):
            nc.sync.dma_start(xts[cb][0][:], xT[cb * P : (cb + 1) * P, 0:1024])
        nc.sync.dma_start(wq_sb[:, :, 0:EG], wqv[:, :, 0:EG])                 # Q
        for cb in range(CB):
            nc.sync.dma_start(xts[cb][1][:], xT[cb * P : (cb + 1) * P, 1024:2048])
        nc.sync.dma_start(wq_sb[:, :, 2 * EG : 3 * EG], wqv[:, :, 2 * EG :])  # V
        wp_sb = persist.tile([P, EG // P, C], bf16, tag="wp")
        nc.sync.dma_start(wp_sb[:], wproj[:].rearrange("(cb p) c -> p cb c", p=P))

        qkT = persist.tile([P, 2 * EG // P, N], bf16, tag="qkT")   # Q blocks 0-2, K 3-5
        # V stationary: per (mt, head): [V_h(64) | ones(1)]
        vst = persist.tile([P, NT, HL, D + 1], bf16, tag="vst")
        nc.vector.memset(vst[:, :, :, D : D + 1], 1.0)
        pones = persist.tile([P, D], bf16, tag="pones")
        nc.vector.memset(pones[:], 1.0)
        ogTs = [persist.tile([P, N], bf16, tag=f"ogT{cb}", name=f"ogT{cb}")
                for cb in range(EG // P)]

        copy_rr = [0]

        def copy_any(dst, src):
            """Alternate psum->sbuf copies between DVE and ACT."""
            copy_rr[0] ^= 1
            if copy_rr[0]:
                nc.vector.tensor_copy(dst, src)
            else:
                nc.scalar.copy(dst, src)

        # ---- QKV units ----
        def qk_unit(eb, hf):
            """One [128,1024] chunk of (QKV)^T block eb, query-half hf."""
            def go():
                psum = ps_sc.tile([P, 1024], f32, tag="sc", name=f"qk{eb}_{hf}")
                for sub in range(2):
                    for cb in range(CB):
                        nc.tensor.matmul(
                            psum[:, sub * 512 : (sub + 1) * 512],
                            wq_sb[:, cb, eb * P : (eb + 1) * P],
                            xts[cb][hf][:, sub * 512 : (sub + 1) * 512],
                            start=(cb == 0),
                            stop=(cb == CB - 1),
                        )
                copy_any(qkT[:, eb, hf * 1024 : (hf + 1) * 1024], psum[:])
            return go

        def v_unit(mt):
            def go():
                vpsum = ps_pv.tile([P, 512], f32, tag="pv", name=f"v{mt}")
                for cb in range(CB):
                    nc.tensor.matmul(
                        vpsum[:, 0:EG],
                        xts[cb][mt // 8][:, (mt % 8) * P : (mt % 8 + 1) * P],
                        wq_sb[:, cb, 2 * EG : 3 * EG],
                        start=(cb == 0),
                        stop=(cb == CB - 1),
                    )
                copy_any(
                    vst[:, mt, :, 0:D],
                    vpsum[:, 0:EG].rearrange("p (h d) -> p h d", d=D),
                )
            return go

        # ---- scores + exp for one head-pair over one query half ----
        def emit_scores_pair(p, hf, work):
            """Per (mt, sub): ONE shared psum tile [128,1024] = [e|o]: head 2p
            (PE rows 0-63) -> cols 0:512, head 2p+1 (rows 64-127) -> cols
            512:1024. Shared tile readiness keeps the e/o matmuls adjacent in
            the schedule so they run concurrently (disjoint row groups).
            exp alternates ACT/DVE per tile; `work` closures are spread
            through the 16 mt steps."""
            kblk = 3 + p
            qblk = p
            pts = [[ptpool.tile([P, 1024], bf16, tag="pt", name=f"pt{p}{hf}_{mt}_{sub}")
                    for sub in range(2)] for mt in range(NT)]
            wi = 0
            for mt in range(NT):
                for sub in range(2):
                    ps = ps_sc.tile([P, 1024], f32, tag="sc", name=f"s{p}{hf}_{mt}_{sub}")
                    off = hf * 1024 + sub * 512
                    nc.tensor.matmul(
                        ps[:, 0:512],
                        qkT[0:D, kblk, mt * P : (mt + 1) * P],
                        qkT[0:D, qblk, off : off + 512], start=True, stop=True,
                    )
                    nc.tensor.matmul(
                        ps[:, 512:1024],
                        qkT[D:P, kblk, mt * P : (mt + 1) * P],
                        qkT[D:P, qblk, off : off + 512], start=True, stop=True,
                    )
                    dve = ((mt * 2 + sub) % 8) < DVE_PER_8
                    if dve:
                        nc.vector.tensor_scalar(
                            pts[mt][sub][:].bitcast(i16), ps[:], EXP_A * SCALE, EXP_B,
                            mybir.AluOpType.mult, mybir.AluOpType.add,
                        )
                    else:
                        nc.scalar.activation(
                            pts[mt][sub][:], ps[:], mybir.ActivationFunctionType.Exp,
                            scale=SCALE,
                        )
                hi = (mt + 1) * len(work) // NT
                while wi < hi:
                    work[wi]()
                    wi += 1
            return pts

        # ---- PV (transposed) + den/normalize chain ----
        def pv_mm(h, hf, qc, pts, pvc):
            """16 accumulating MMs: outT_h rows 0-63 + den row 64, then copy
            psum -> sbuf staging (frees the psum bank fast). Queries qc*512..
            of this half live in pts[mt][qc], cols (h%2)*512..+512."""
            def go():
                ho = (h % 2) * 512
                pvp = ps_pv.tile([P, 512], f32, tag="pv", name=f"pv{h}{hf}{qc}")
                for mt in range(NT):
                    nc.tensor.matmul(
                        pvp[0 : D + 1, :],
                        vst[:, mt, h, :],
                        pts[mt][qc][:, ho : ho + 512],
                        start=(mt == 0),
                        stop=(mt == NT - 1),
                    )
                copy_any(pvc[0 : D + 1, qc * 512 : (qc + 1) * 512],
                         pvp[0 : D + 1, :])
            return go

        def den_norm(h, hf, qa, qb):
            """den cols [qa*512, qb*512) -> DRAM -> compact recip -> DRAM ->
            bcast [64, L] -> normalize into ogT. Batched (qa,qb)=(0,2) for
            steady state; per-qc for the tail pair."""
            def go():
                j = h * 2 + hf
                L = (qb - qa) * 512
                qs = slice(qa * 512, qb * 512)
                nc.gpsimd.dma_start(
                    den_d[j, qs].rearrange("(o q) -> o q", o=1), pvc_of[h][D : D + 1, qs]
                )
                denc = dencpool.tile([P, 8], bf16, tag="dc", name=f"dc{j}_{qa}")
                nc.gpsimd.dma_start(
                    denc[:, 0 : L // P], den_d[j, qs].rearrange("(p f) -> p f", p=P)
                )
                rcc = dencpool.tile([P, 8], bf16, tag="dc", name=f"rc{j}_{qa}")
                with nc.allow_low_precision(reason="bf16 softmax 1/den"):
                    nc.vector.reciprocal(rcc[:, 0 : L // P], denc[:, 0 : L // P])
                nc.gpsimd.dma_start(
                    rcp_d[j, qs].rearrange("(p f) -> p f", p=P), rcc[:, 0 : L // P]
                )
                rcpb = rcppool.tile([D, 1024], bf16, tag="rb", name=f"rb{j}_{qa}")
                nc.gpsimd.dma_start(
                    rcpb[:, 0:L],
                    rcp_d[j, qs].rearrange("q -> () q").broadcast_to([D, L]),
                )
                cb, half = h // 2, h % 2
                nc.vector.tensor_tensor(
                    ogTs[cb][half * D : (half + 1) * D,
                             hf * 1024 + qa * 512 : hf * 1024 + qb * 512],
                    pvc_of[h][0:D, qs], rcpb[:, 0:L], mybir.AluOpType.mult,
                )
            return go

        # bf16 magic-constant reciprocal + one Newton step (tail only):
        # den row -> PE broadcast [64,512] -> r0 = bitcast(0x7EF3 - bits(d))
        # -> r1 = r0*(2 - d*r0) -> normalize. ~2.5us latency, no DMA.
        RCP_MAGIC = float(0x7EF3)

        def den_newton(h, hf, qc):
            def go():
                qs = slice(qc * 512, (qc + 1) * 512)
                pvc = pvc_of[h]
                denb = ps_pv.tile([P, 512], f32, tag="pv", name=f"dnb{h}{qc}")
                nc.tensor.matmul(denb[0:D, :], pones[64:65, :], pvc[64:65, qs],
                                 start=True, stop=True)
                dbs = nwtpool.tile([D, 512], bf16, tag="nw", name=f"dbs{h}{qc}")
                r0 = nwtpool.tile([D, 512], bf16, tag="nw", name=f"r0{h}{qc}")
                tt = nwtpool.tile([D, 512], bf16, tag="nw", name=f"tt{h}{qc}")
                nc.scalar.copy(dbs[:], denb[0:D, :])
                nc.vector.tensor_scalar(
                    r0[:].bitcast(i16), dbs[:].bitcast(i16), -1.0, RCP_MAGIC,
                    mybir.AluOpType.mult, mybir.AluOpType.add,
                )
                nc.vector.tensor_tensor(tt[:], dbs[:], r0[:], mybir.AluOpType.mult)
                nc.vector.tensor_scalar(dbs[:], tt[:], -1.0, 2.0,
                                        mybir.AluOpType.mult, mybir.AluOpType.add)
                nc.vector.tensor_tensor(tt[:], r0[:], dbs[:], mybir.AluOpType.mult)
                cb, half = h // 2, h % 2
                nc.vector.tensor_tensor(
                    ogTs[cb][half * D : (half + 1) * D,
                             hf * 1024 + qc * 512 : hf * 1024 + (qc + 1) * 512],
                    pvc[0:D, qs], tt[:], mybir.AluOpType.mult,
                )
            return go

        pvc_of = {}

        def pv_pair(p, hf, pts, split_den=False):
            """Work closures for both heads of pair p on half hf."""
            units = []
            for h in (2 * p, 2 * p + 1):
                pvc_of[h] = pvcpool.tile([D + 1, 1024], bf16, tag="pvc",
                                         name=f"pvc{h}{hf}")
                pvc = pvc_of[h]
                if split_den:
                    for qc in range(2):
                        units.append(pv_mm(h, hf, qc, pts, pvc))
                        units.append(den_newton(h, hf, qc))
                else:
                    units.append(pv_mm(h, hf, 0, pts, pvc))
                    units.append(pv_mm(h, hf, 1, pts, pvc))
                    units.append(den_norm(h, hf, 0, 2))
            return units

        # ---- proj ----
        yv = y[:].rearrange("(nt p) c -> p nt c", p=P)

        def proj_unit(nt):
            def go():
                y_sb = ypool.tile([P, C], bf16, tag="y", name=f"y{nt}")
                for halfc in range(2):
                    ppsum = ps_pv.tile([P, 512], f32, tag="pv", name=f"pj{nt}_{halfc}")
                    for cb in range(EG // P):
                        nc.tensor.matmul(
                            ppsum[:, 0:EG],
                            ogTs[cb][:, nt * P : (nt + 1) * P],
                            wp_sb[:, cb, halfc * EG : (halfc + 1) * EG],
                            start=(cb == 0),
                            stop=(cb == EG // P - 1),
                        )
                    copy_any(y_sb[:, halfc * EG : (halfc + 1) * EG], ppsum[:, 0:EG])
                nc.sync.dma_start(yv[:, nt, :], y_sb[:])
            return go

        # ---- emission schedule ----
        # Phases hf-major: (p, hf) = 00,10,20,01,11,21. Pair p's scores need
        # K block (3+p) over both key halves and Q block p for half hf.
        # PV of phase i rides phase i+1's score emission.
        qk_unit(3, 0)()
        qk_unit(3, 1)()
        qk_unit(0, 0)()
        pending = {
            0: [qk_unit(4, 0), qk_unit(4, 1), qk_unit(1, 0)]
               + [v_unit(m) for m in range(NT)],
            1: [qk_unit(5, 0), qk_unit(5, 1), qk_unit(2, 0)],
            2: [qk_unit(0, 1)],
            3: [qk_unit(1, 1)],
            4: [qk_unit(2, 1)] + [proj_unit(nt) for nt in range(6)],
            5: [proj_unit(6), proj_unit(7)],
        }
        order = [(0, 0), (1, 0), (2, 0), (0, 1), (1, 1), (2, 1)]
        prev_pv = []
        for i, (p, hf) in enumerate(order):
            work = list(prev_pv) + pending.get(i, [])
            pts = emit_scores_pair(p, hf, work)
            prev_pv = pv_pair(p, hf, pts, split_den=(i == 5))
        # tail: last pair's PV qc0 bursts + chains first, then qc1, then
        # warm-keeper spins (bridge the den-chain latency without letting
        # the PE HAM re-throttle), then proj of half 1.
        e0, de0, e1, de1, o0, do0, o1, do1 = prev_pv
        e0(); de0(); o0(); do0()
        e1(); de1(); o1(); do1()
        spin_ps = ps_sc.tile([P, 64], f32, tag="sc", name="tailspin")
        for w in range(TAIL_SPINS):
            nc.tensor.matmul(spin_ps[:, 0:32], junk[:], junk[:, 0:32],
                             start=True, stop=True)
        for nt in range(8, NT):
            proj_unit(nt)()

    nc.compile()
    return nc


_PROGRAM = None


def _get_program():
    global _PROGRAM
    if _PROGRAM is None:
        _PROGRAM = _build_program()
    return _PROGRAM


def _shard_inputs(x, Wqkv, Wproj):
    bf = ml_dtypes.bfloat16
    in_maps = []
    for core in range(NCORES):
        b, g = core // G, core % G
        xT = np.ascontiguousarray(x[b].T).astype(bf)
        wg = np.concatenate(
            [
                Wqkv[:, g * EG : (g + 1) * EG],
                Wqkv[:, C + g * EG : C + (g + 1) * EG],
                Wqkv[:, 2 * C + g * EG : 2 * C + (g + 1) * EG],
            ],
            axis=1,
        ).astype(bf)
        wp = np.ascontiguousarray(Wproj[g * EG : (g + 1) * EG, :]).astype(bf)
        in_maps.append({"xT": xT, "wqkv": wg, "wproj": wp})
    return in_maps


def _run(x, Wqkv, Wproj, bproj, trace=False):
    nc = _get_program()
    in_maps = _shard_inputs(x, Wqkv, Wproj)
    res = run_bass_kernel_spmd(nc, in_maps, list(range(NCORES)), trace=trace)
    out = np.empty((B, N, C), np.float32)
    for b in range(B):
        out[b] = (
            res.results[b * G]["y"].astype(np.float32)
            + res.results[b * G + 1]["y"].astype(np.float32)
            + bproj
        )
    return out, res


def kernel(x, Wqkv, Wproj, bproj):
    x = np.asarray(x, np.float32)
    Wqkv = np.asarray(Wqkv, np.float32)
    Wproj = np.asarray(Wproj, np.float32)
    bproj = np.asarray(bproj, np.float32)
    out, _ = _run(x, Wqkv, Wproj, bproj)
    return out


# revision 18
# speedup vs baseline: 1.1937x; 1.0195x over previous
"""Multi-head attention block for Trainium2, SPMD over 8 NeuronCores. v3.

Sharding: 8 shards = batch (4) x head-group (2 groups of 6 heads).
Per core (b, g), for its 6 heads:
    qkv   = x[b] @ Wqkv[:, cols(g)]            (bf16 matmul, fp32 accum)
    S^T_h = K_h Q_h^T   per head               (keys on partitions)
      - heads are processed in PAIRS (2p, 2p+1) whose K/Q blocks sit at
        partition rows 0-63 / 64-127, so consecutive score matmuls target
        disjoint PE row-groups and run concurrently (2x tensor throughput
        for the K=64 contraction).
    P^T_h = exp(SCALE * S^T_h)  -> bf16
      - score tiles alternate between the ACT engine (table exp, exact)
        and the DVE (Schraudolph int16 bit-trick exp, ~2% rms), splitting
        the ~25M exp/core across both element-wise engines.
    out_h = (P_h @ [V_h | 1]) -> normalize rows by the ones-column sum
    y_g   = concat_h(out_h) @ Wproj[rows(g), :]    (partial, fp32 out)
Host sums the two head-group partials per batch and adds bproj.

Queries are processed in halves of 1024 ([128,1024] exp ops amortize the
PSUM-access overhead); PV of each phase rides the next phase's score
emission to keep the PE dense.

Shapes hardcoded: x [4, 2048, 768], Wqkv [768, 2304], Wproj [768, 768].
"""

import os
from contextlib import ExitStack

import numpy as np
import ml_dtypes

import concourse.bass as bass
import concourse.mybir as mybir
import concourse.tile as tile
from concourse import bacc
from concourse.bass_utils import run_bass_kernel_spmd
from concourse.masks import make_identity

B, N, C = 4, 2048, 768
H, D = 12, 64          # total heads, head dim
G = 2                  # head groups (tensor-parallel axis)
HL = H // G            # heads per core = 6
SCALE = D ** -0.5
P = 128
CB = C // P            # 6 contraction blocks
NT = N // P            # 16 row tiles
NHALF = N // 1024      # 2 query halves
EG = HL * D            # 384 = per-group width of Q / K / V
NCORES = 8

f32 = mybir.dt.float32
bf16 = mybir.dt.bfloat16
i16 = mybir.dt.int16

# Schraudolph bf16 exp: exp(x) ~= bitcast_bf16(int16(x * 128/ln2 + B))
EXP_A = float(128.0 / np.log(2.0))
EXP_B = 16250.5

DVE_PER_8 = int(os.environ.get("KRN_DVE_PER_8", "3"))  # of 8 exp tiles -> DVE
PT_BUFS = int(os.environ.get("KRN_PT_BUFS", "58"))


def _build_program():
    nc = bacc.Bacc("TRN2", target_bir_lowering=False, debug=False)

    xT = nc.dram_tensor("xT", [C, N], bf16, kind="ExternalInput")           # x[b].T
    wqkv = nc.dram_tensor("wqkv", [C, 3 * EG], bf16, kind="ExternalInput")  # [Qg|Kg|Vg]
    wproj = nc.dram_tensor("wproj", [EG, C], bf16, kind="ExternalInput")    # group rows
    y = nc.dram_tensor("y", [N, C], f32, kind="ExternalOutput")             # partial out

    with tile.TileContext(nc) as tc, ExitStack() as ctx:
        persist = ctx.enter_context(tc.tile_pool(name="persist", bufs=1))
        ptpool = ctx.enter_context(tc.tile_pool(name="ptpool", bufs=PT_BUFS))
        ypool = ctx.enter_context(tc.tile_pool(name="ypool", bufs=3))
        ps_sc = ctx.enter_context(tc.tile_pool(name="ps_sc", bufs=3, space="PSUM"))
        ps_sm = ctx.enter_context(tc.tile_pool(name="ps_sm", bufs=2, space="PSUM"))

        identity = persist.tile([P, P], bf16, tag="identity")
        make_identity(nc, identity)

        # PE clock warm-up spin + ACT exp-table preload, overlapping the
        # input DMA window (identity is generated on-chip, no DMA deps).
        warm_sb = persist.tile([P, 16], bf16, tag="warm")
        nc.scalar.activation(warm_sb[:], identity[:, :16],
                             mybir.ActivationFunctionType.Exp)
        for w in range(40):
            wps = ps_sc.tile([P, P], f32, tag="sc", name=f"warm{w}")
            nc.tensor.matmul(wps[:], identity[:], identity[:],
                             start=True, stop=True)

        # ---- persistent tiles ----
        # Load order matters for the warmup: K columns + x half-0 first so
        # the first score-feeding QKV chunks start ~7us in, not ~18us.
        wq_sb = persist.tile([P, CB, 3 * EG], bf16, tag="wq")
        wqv = wqkv[:].rearrange("(cb p) e -> p cb e", p=P)
        nc.sync.dma_start(wq_sb[:, :, EG : 2 * EG], wqv[:, :, EG : 2 * EG])   # K
        xts = [[ptpool.tile([P, 1024], bf16, tag="pt", name=f"xt{cb}_{hf}")
                for hf in range(NHALF)] for cb in range(CB)]
        for cb in range(CB):
            nc.sync.dma_start(xts[cb][0][:], xT[cb * P : (cb + 1) * P, 0:1024])
        nc.sync.dma_start(wq_sb[:, :, 0:EG], wqv[:, :, 0:EG])                 # Q
        for cb in range(CB):
            nc.sync.dma_start(xts[cb][1][:], xT[cb * P : (cb + 1) * P, 1024:2048])
        nc.sync.dma_start(wq_sb[:, :, 2 * EG : 3 * EG], wqv[:, :, 2 * EG :])  # V
        wp_sb = persist.tile([P, EG // P, C], bf16, tag="wp")
        nc.sync.dma_start(wp_sb[:], wproj[:].rearrange("(cb p) c -> p cb c", p=P))

        qkT = persist.tile([P, 2 * EG // P, N], bf16, tag="qkT")   # Q blocks 0-2, K 3-5
        vp_sb = persist.tile([P, NT, HL * (D + 1)], bf16, tag="vp")
        vp4 = vp_sb.rearrange("p m (h c) -> p m h c", c=D + 1)
        nc.vector.memset(vp4[:, :, :, D : D + 1], 1.0)
        og_sb = persist.tile([P, NT, EG], bf16, tag="og")          # heads out [n, ch]
        ogTs = [persist.tile([P, N], bf16, tag=f"ogT{cb}", name=f"ogT{cb}")
                for cb in range(EG // P)]
        rr = persist.tile([P, HL, NT], f32, tag="rr")              # 1/den per head,nt

        copy_rr = [0]

        def copy_any(dst, src):
            """Alternate psum->sbuf copies between DVE and ACT."""
            copy_rr[0] ^= 1
            if copy_rr[0]:
                nc.vector.tensor_copy(dst, src)
            else:
                nc.scalar.copy(dst, src)

        # ---- QKV units ----
        def qk_unit(eb, hf):
            """One [128,1024] chunk of (QKV)^T block eb, query-half hf."""
            def go():
                psum = ps_sc.tile([P, 1024], f32, tag="sc", name=f"qk{eb}_{hf}")
                for sub in range(2):
                    for cb in range(CB):
                        nc.tensor.matmul(
                            psum[:, sub * 512 : (sub + 1) * 512],
                            wq_sb[:, cb, eb * P : (eb + 1) * P],
                            xts[cb][hf][:, sub * 512 : (sub + 1) * 512],
                            start=(cb == 0),
                            stop=(cb == CB - 1),
                        )
                copy_any(qkT[:, eb, hf * 1024 : (hf + 1) * 1024], psum[:])
            return go

        def v_unit(mt):
            def go():
                vpsum = ps_sm.tile([P, EG], f32, tag="sm", name=f"v{mt}")
                for cb in range(CB):
                    nc.tensor.matmul(
                        vpsum[:],
                        xts[cb][mt // 8][:, (mt % 8) * P : (mt % 8 + 1) * P],
                        wq_sb[:, cb, 2 * EG : 3 * EG],
                        start=(cb == 0),
                        stop=(cb == CB - 1),
                    )
                copy_any(
                    vp4[:, mt, :, :D],
                    vpsum[:].rearrange("p (h d) -> p h d", d=D),
                )
            return go

        # ---- scores + exp for one head-pair over one query half ----
        def emit_scores_pair(p, hf, work):
            """Per mt: score matmuls for head 2p (PE rows 0-63) and 2p+1
            (rows 64-127) run concurrently; exp alternates ACT/DVE.
            `work` closures are spread through the 16 mt steps."""
            kblk = 3 + p
            qblk = p
            pts_e = [ptpool.tile([P, 1024], bf16, tag="pt", name=f"pe{p}{hf}_{mt}")
                     for mt in range(NT)]
            pts_o = [ptpool.tile([P, 1024], bf16, tag="pt", name=f"po{p}{hf}_{mt}")
                     for mt in range(NT)]
            wi = 0
            for mt in range(NT):
                pse = ps_sc.tile([P, 1024], f32, tag="sc", name=f"se{p}{hf}_{mt}")
                pso = ps_sc.tile([P, 1024], f32, tag="sc", name=f"so{p}{hf}_{mt}")
                for sub in range(2):
                    off = hf * 1024 + sub * 512
                    nc.tensor.matmul(
                        pse[:, sub * 512 : (sub + 1) * 512],
                        qkT[0:D, kblk, mt * P : (mt + 1) * P],
                        qkT[0:D, qblk, off : off + 512], start=True, stop=True,
                    )
                    nc.tensor.matmul(
                        pso[:, sub * 512 : (sub + 1) * 512],
                        qkT[D:P, kblk, mt * P : (mt + 1) * P],
                        qkT[D:P, qblk, off : off + 512], start=True, stop=True,
                    )
                dve_first = (mt % 8) < DVE_PER_8
                for pts, ps, dve in ((pts_e, pse, dve_first),
                                     (pts_o, pso, not dve_first)):
                    if dve:
                        nc.vector.tensor_scalar(
                            pts[mt][:].bitcast(i16), ps[:], EXP_A * SCALE, EXP_B,
                            mybir.AluOpType.mult, mybir.AluOpType.add,
                        )
                    else:
                        nc.scalar.activation(
                            pts[mt][:], ps[:], mybir.ActivationFunctionType.Exp,
                            scale=SCALE,
                        )
                hi = (mt + 1) * len(work) // NT
                while wi < hi:
                    work[wi]()
                    wi += 1
            return pts_e, pts_o

        # ---- PV for one head over 4 query blocks (512 q, one psum bank) ----
        def pv_quad(h, hf, q4, pts):
            def go():
                pvps = ps_sm.tile([P, 4, D + 1], f32, tag="sm", name=f"pv{h}{hf}{q4}")
                for k in range(4):
                    kq = q4 * 4 + k
                    for mt in range(NT):
                        nc.tensor.matmul(
                            pvps[:, k, :],
                            pts[mt][:, kq * P : (kq + 1) * P],
                            vp4[:, mt, h, :],
                            start=(mt == 0),
                            stop=(mt == NT - 1),
                        )
                nt0 = hf * 8 + q4 * 4
                nc.vector.reciprocal(rr[:, h, nt0 : nt0 + 4], pvps[:, :, D])
                # batched normalize: one DVE op over all 4 nt blocks, with
                # 1/den broadcast along a 0-stride 64-wide dim
                rb = (rr[:, h, nt0 : nt0 + 4]
                      .rearrange("p f -> p f ()").broadcast_to([P, 4, D]))
                nc.vector.tensor_tensor(
                    og_sb[:, nt0 : nt0 + 4, h * D : (h + 1) * D],
                    pvps[:, :, :D], rb, mybir.AluOpType.mult,
                )
            return go

        def pv_units(p, hf, pts_e, pts_o):
            return [pv_quad(2 * p, hf, 0, pts_e), pv_quad(2 * p + 1, hf, 0, pts_o),
                    pv_quad(2 * p, hf, 1, pts_e), pv_quad(2 * p + 1, hf, 1, pts_o)]

        # ---- og transposes + proj ----
        def ogT_unit(cb, nt, pool=None):
            def go():
                tpsum = (pool or ps_sm).tile([P, P], bf16,
                                             tag="sm" if pool is None else "sc",
                                             name=f"t{cb}_{nt}")
                nc.tensor.transpose(
                    tpsum[:], og_sb[:, nt, cb * P : (cb + 1) * P], identity
                )
                copy_any(ogTs[cb][:, nt * P : (nt + 1) * P], tpsum[:])
            return go

        yv = y[:].rearrange("(nt p) c -> p nt c", p=P)

        def proj_unit(nt):
            def go():
                y_sb = ypool.tile([P, C], f32, tag="y", name=f"y{nt}")
                ppsum = ps_sc.tile([P, 1024], f32, tag="sc", name=f"pj{nt}")
                for halfc in range(2):
                    for cb in range(EG // P):
                        nc.tensor.matmul(
                            ppsum[:, halfc * 512 : halfc * 512 + EG],
                            ogTs[cb][:, nt * P : (nt + 1) * P],
                            wp_sb[:, cb, halfc * EG : (halfc + 1) * EG],
                            start=(cb == 0),
                            stop=(cb == EG // P - 1),
                        )
                    copy_any(
                        y_sb[:, halfc * EG : (halfc + 1) * EG],
                        ppsum[:, halfc * 512 : halfc * 512 + EG],
                    )
                    nc.sync.dma_start(
                        yv[:, nt, halfc * EG : (halfc + 1) * EG],
                        y_sb[:, halfc * EG : (halfc + 1) * EG],
                    )
            return go

        # ---- emission schedule ----
        # Phase i = (pair p=i//2, half hf=i%2). Pair p's scores need its
        # K block (3+p) over BOTH key halves, and Q block p for half hf —
        # emitted at least one phase ahead.
        qk_unit(3, 0)()
        qk_unit(3, 1)()
        qk_unit(0, 0)()
        pending = {
            0: [v_unit(m) for m in range(NT)] + [qk_unit(0, 1)],
            1: [qk_unit(4, 0), qk_unit(4, 1), qk_unit(1, 0)],
            2: [qk_unit(1, 1)],
            3: [qk_unit(5, 0), qk_unit(5, 1), qk_unit(2, 0)]
               + [ogT_unit(0, nt) for nt in range(NT)],
            4: [qk_unit(2, 1)],
            5: [ogT_unit(1, nt) for nt in range(NT)],
        }
        prev_pv = []
        for i in range(3 * NHALF):
            p, hf = i // NHALF, i % NHALF
            work = list(prev_pv) + pending.get(i, [])
            pts_e, pts_o = emit_scores_pair(p, hf, work)
            prev_pv = pv_units(p, hf, pts_e, pts_o)
        # tail: ogT2/proj for the first half (whose PV is already emitted)
        # interleave with the last pair's second-half PV, then the rest.
        ot2 = [ogT_unit(2, nt, pool=ps_sc) for nt in range(NT)]
        for nt in range(4):
            ot2[nt]()
        prev_pv[0]()
        prev_pv[1]()
        for nt in range(4, 8):
            ot2[nt]()
        for nt in range(4):
            proj_unit(nt)()
        prev_pv[2]()
        prev_pv[3]()
        for nt in range(4, 8):
            proj_unit(nt)()
        for nt in range(8, NT):
            ot2[nt]()
            proj_unit(nt)()

    nc.compile()
    return nc


_PROGRAM = None


def _get_program():
    global _PROGRAM
    if _PROGRAM is None:
        _PROGRAM = _build_program()
    return _PROGRAM


def _shard_inputs(x, Wqkv, Wproj):
    bf = ml_dtypes.bfloat16
    in_maps = []
    for core in range(NCORES):
        b, g = core // G, core % G
        xT = np.ascontiguousarray(x[b].T).astype(bf)
        wg = np.concatenate(
            [
                Wqkv[:, g * EG : (g + 1) * EG],
                Wqkv[:, C + g * EG : C + (g + 1) * EG],
                Wqkv[:, 2 * C + g * EG : 2 * C + (g + 1) * EG],
            ],
            axis=1,
        ).astype(bf)
        wp = np.ascontiguousarray(Wproj[g * EG : (g + 1) * EG, :]).astype(bf)
        in_maps.append({"xT": xT, "wqkv": wg, "wproj": wp})
    return in_maps


def _run(x, Wqkv, Wproj, bproj, trace=False):
    nc = _get_program()
    in_maps = _shard_inputs(x, Wqkv, Wproj)
    res = run_bass_kernel_spmd(nc, in_maps, list(range(NCORES)), trace=trace)
    out = np.empty((B, N, C), np.float32)
    for b in range(B):
        out[b] = res.results[b * G]["y"] + res.results[b * G + 1]["y"] + bproj
    return out, res


def kernel(x, Wqkv, Wproj, bproj):
    x = np.asarray(x, np.float32)
    Wqkv = np.asarray(Wqkv, np.float32)
    Wproj = np.asarray(Wproj, np.float32)
    bproj = np.asarray(bproj, np.float32)
    out, _ = _run(x, Wqkv, Wproj, bproj)
    return out



# revision 19
# speedup vs baseline: 1.3122x; 1.0993x over previous
"""Multi-head attention block for Trainium2, SPMD over 8 NeuronCores. v3.

Sharding: 8 shards = batch (4) x head-group (2 groups of 6 heads).
Per core (b, g), for its 6 heads:
    qkv   = x[b] @ Wqkv[:, cols(g)]            (bf16 matmul, fp32 accum)
    S^T_h = K_h Q_h^T   per head               (keys on partitions)
      - heads are processed in PAIRS (2p, 2p+1) whose K/Q blocks sit at
        partition rows 0-63 / 64-127, so consecutive score matmuls target
        disjoint PE row-groups and run concurrently (2x tensor throughput
        for the K=64 contraction).
    P^T_h = exp(SCALE * S^T_h)  -> bf16
      - score tiles alternate between the ACT engine (table exp, exact)
        and the DVE (Schraudolph int16 bit-trick exp, ~2% rms), splitting
        the ~25M exp/core across both element-wise engines.
    out_h = (P_h @ [V_h | 1]) -> normalize rows by the ones-column sum
    y_g   = concat_h(out_h) @ Wproj[rows(g), :]    (partial, fp32 out)
Host sums the two head-group partials per batch and adds bproj.

Queries are processed in halves of 1024 ([128,1024] exp ops amortize the
PSUM-access overhead); PV of each phase rides the next phase's score
emission to keep the PE dense.

Shapes hardcoded: x [4, 2048, 768], Wqkv [768, 2304], Wproj [768, 768].
"""

import os
from contextlib import ExitStack

import numpy as np
import ml_dtypes

import concourse.bass as bass
import concourse.mybir as mybir
import concourse.tile as tile
from concourse import bacc
from concourse.bass_utils import run_bass_kernel_spmd
from concourse.masks import make_identity

B, N, C = 4, 2048, 768
H, D = 12, 64          # total heads, head dim
G = 2                  # head groups (tensor-parallel axis)
HL = H // G            # heads per core = 6
SCALE = D ** -0.5
P = 128
CB = C // P            # 6 contraction blocks
NT = N // P            # 16 row tiles
NHALF = N // 1024      # 2 query halves
EG = HL * D            # 384 = per-group width of Q / K / V
NCORES = 8

f32 = mybir.dt.float32
bf16 = mybir.dt.bfloat16
i16 = mybir.dt.int16

# Schraudolph bf16 exp: exp(x) ~= bitcast_bf16(int16(x * 128/ln2 + B))
EXP_A = float(128.0 / np.log(2.0))
EXP_B = 16250.5

DVE_PER_8 = int(os.environ.get("KRN_DVE_PER_8", "3"))  # of 8 exp tiles -> DVE
PT_BUFS = int(os.environ.get("KRN_PT_BUFS", "58"))


def _build_program():
    nc = bacc.Bacc("TRN2", target_bir_lowering=False, debug=False)

    xT = nc.dram_tensor("xT", [C, N], bf16, kind="ExternalInput")           # x[b].T
    wqkv = nc.dram_tensor("wqkv", [C, 3 * EG], bf16, kind="ExternalInput")  # [Qg|Kg|Vg]
    wproj = nc.dram_tensor("wproj", [EG, C], bf16, kind="ExternalInput")    # group rows
    y = nc.dram_tensor("y", [N, C], f32, kind="ExternalOutput")             # partial out

    with tile.TileContext(nc) as tc, ExitStack() as ctx:
        persist = ctx.enter_context(tc.tile_pool(name="persist", bufs=1))
        ptpool = ctx.enter_context(tc.tile_pool(name="ptpool", bufs=PT_BUFS))
        ypool = ctx.enter_context(tc.tile_pool(name="ypool", bufs=3))
        ps_sc = ctx.enter_context(tc.tile_pool(name="ps_sc", bufs=3, space="PSUM"))
        ps_sm = ctx.enter_context(tc.tile_pool(name="ps_sm", bufs=2, space="PSUM"))

        identity = persist.tile([P, P], bf16, tag="identity")
        make_identity(nc, identity)

        # PE clock warm-up spin + ACT exp-table preload, overlapping the
        # input DMA window (identity is generated on-chip, no DMA deps).
        warm_sb = persist.tile([P, 16], bf16, tag="warm")
        nc.scalar.activation(warm_sb[:], identity[:, :16],
                             mybir.ActivationFunctionType.Exp)
        for w in range(40):
            wps = ps_sc.tile([P, P], f32, tag="sc", name=f"warm{w}")
            nc.tensor.matmul(wps[:], identity[:], identity[:],
                             start=True, stop=True)

        # ---- persistent tiles ----
        # Load order matters for the warmup: K columns + x half-0 first so
        # the first score-feeding QKV chunks start ~7us in, not ~18us.
        wq_sb = persist.tile([P, CB, 3 * EG], bf16, tag="wq")
        wqv = wqkv[:].rearrange("(cb p) e -> p cb e", p=P)
        nc.sync.dma_start(wq_sb[:, :, EG : 2 * EG], wqv[:, :, EG : 2 * EG])   # K
        xts = [[ptpool.tile([P, 1024], bf16, tag="pt", name=f"xt{cb}_{hf}")
                for hf in range(NHALF)] for cb in range(CB)]
        for cb in range(CB):
            nc.sync.dma_start(xts[cb][0][:], xT[cb * P : (cb + 1) * P, 0:1024])
        nc.sync.dma_start(wq_sb[:, :, 0:EG], wqv[:, :, 0:EG])                 # Q
        for cb in range(CB):
            nc.sync.dma_start(xts[cb][1][:], xT[cb * P : (cb + 1) * P, 1024:2048])
        nc.sync.dma_start(wq_sb[:, :, 2 * EG : 3 * EG], wqv[:, :, 2 * EG :])  # V
        wp_sb = persist.tile([P, EG // P, C], bf16, tag="wp")
        nc.sync.dma_start(wp_sb[:], wproj[:].rearrange("(cb p) c -> p cb c", p=P))

        qkT = persist.tile([P, 2 * EG // P, N], bf16, tag="qkT")   # Q blocks 0-2, K 3-5
        vp_sb = persist.tile([P, NT, HL * (D + 1)], bf16, tag="vp")
        vp4 = vp_sb.rearrange("p m (h c) -> p m h c", c=D + 1)
        nc.vector.memset(vp4[:, :, :, D : D + 1], 1.0)
        og_sb = persist.tile([P, NT, EG], bf16, tag="og")          # heads out [n, ch]
        ogTs = [persist.tile([P, N], bf16, tag=f"ogT{cb}", name=f"ogT{cb}")
                for cb in range(EG // P)]
        rr = persist.tile([P, HL, NT], f32, tag="rr")              # 1/den per head,nt

        copy_rr = [0]

        def copy_any(dst, src):
            """Alternate psum->sbuf copies between DVE and ACT."""
            copy_rr[0] ^= 1
            if copy_rr[0]:
                nc.vector.tensor_copy(dst, src)
            else:
                nc.scalar.copy(dst, src)

        # ---- QKV units ----
        def qk_unit(eb, hf):
            """One [128,1024] chunk of (QKV)^T block eb, query-half hf."""
            def go():
                psum = ps_sc.tile([P, 1024], f32, tag="sc", name=f"qk{eb}_{hf}")
                for sub in range(2):
                    for cb in range(CB):
                        nc.tensor.matmul(
                            psum[:, sub * 512 : (sub + 1) * 512],
                            wq_sb[:, cb, eb * P : (eb + 1) * P],
                            xts[cb][hf][:, sub * 512 : (sub + 1) * 512],
                            start=(cb == 0),
                            stop=(cb == CB - 1),
                        )
                copy_any(qkT[:, eb, hf * 1024 : (hf + 1) * 1024], psum[:])
            return go

        def v_unit(mt):
            def go():
                vpsum = ps_sm.tile([P, EG], f32, tag="sm", name=f"v{mt}")
                for cb in range(CB):
                    nc.tensor.matmul(
                        vpsum[:],
                        xts[cb][mt // 8][:, (mt % 8) * P : (mt % 8 + 1) * P],
                        wq_sb[:, cb, 2 * EG : 3 * EG],
                        start=(cb == 0),
                        stop=(cb == CB - 1),
                    )
                copy_any(
                    vp4[:, mt, :, :D],
                    vpsum[:].rearrange("p (h d) -> p h d", d=D),
                )
            return go

        # ---- scores + exp for one head-pair over one query half ----
        def emit_scores_pair(p, hf, work):
            """Per (mt, sub): ONE shared psum tile [128,1024] = [e|o]: head 2p
            (PE rows 0-63) -> cols 0:512, head 2p+1 (rows 64-127) -> cols
            512:1024. Shared-tile readiness keeps the e/o matmuls adjacent in
            the schedule so they run concurrently (disjoint row groups).
            exp alternates ACT/DVE per tile; `work` closures are spread
            through the 16 mt steps."""
            kblk = 3 + p
            qblk = p
            pts = [[ptpool.tile([P, 1024], bf16, tag="pt", name=f"pt{p}{hf}_{mt}_{sub}")
                    for sub in range(2)] for mt in range(NT)]
            wi = 0
            for mt in range(NT):
                for sub in range(2):
                    ps = ps_sc.tile([P, 1024], f32, tag="sc", name=f"s{p}{hf}_{mt}_{sub}")
                    off = hf * 1024 + sub * 512
                    nc.tensor.matmul(
                        ps[:, 0:512],
                        qkT[0:D, kblk, mt * P : (mt + 1) * P],
                        qkT[0:D, qblk, off : off + 512], start=True, stop=True,
                    )
                    nc.tensor.matmul(
                        ps[:, 512:1024],
                        qkT[D:P, kblk, mt * P : (mt + 1) * P],
                        qkT[D:P, qblk, off : off + 512], start=True, stop=True,
                    )
                    if (mt * 2 + sub) % 2 == 0:
                        nc.vector.tensor_scalar(
                            pts[mt][sub][:].bitcast(i16), ps[:], EXP_A * SCALE, EXP_B,
                            mybir.AluOpType.mult, mybir.AluOpType.add,
                        )
                    else:
                        nc.scalar.activation(
                            pts[mt][sub][:], ps[:], mybir.ActivationFunctionType.Exp,
                            scale=SCALE,
                        )
                hi = (mt + 1) * len(work) // NT
                while wi < hi:
                    work[wi]()
                    wi += 1
            return pts

        # ---- PV for one head over 4 query blocks (512 q, one psum bank) ----
        def pv_quad(h, hf, q4, pts):
            def go():
                pvps = ps_sm.tile([P, 4, D + 1], f32, tag="sm", name=f"pv{h}{hf}{q4}")
                ho = (h % 2) * 512
                for k in range(4):
                    kq = q4 * 4 + k
                    sub, qo = kq // 4, (kq % 4) * P
                    for mt in range(NT):
                        nc.tensor.matmul(
                            pvps[:, k, :],
                            pts[mt][sub][:, ho + qo : ho + qo + P],
                            vp4[:, mt, h, :],
                            start=(mt == 0),
                            stop=(mt == NT - 1),
                        )
                nt0 = hf * 8 + q4 * 4
                nc.vector.reciprocal(rr[:, h, nt0 : nt0 + 4], pvps[:, :, D])
                # batched normalize: one DVE op over all 4 nt blocks, with
                # 1/den broadcast along a 0-stride 64-wide dim
                rb = (rr[:, h, nt0 : nt0 + 4]
                      .rearrange("p f -> p f ()").broadcast_to([P, 4, D]))
                nc.vector.tensor_tensor(
                    og_sb[:, nt0 : nt0 + 4, h * D : (h + 1) * D],
                    pvps[:, :, :D], rb, mybir.AluOpType.mult,
                )
            return go

        def pv_units(p, hf, pts):
            return [pv_quad(2 * p, hf, 0, pts), pv_quad(2 * p + 1, hf, 0, pts),
                    pv_quad(2 * p, hf, 1, pts), pv_quad(2 * p + 1, hf, 1, pts)]

        # ---- og transposes + proj ----
        def ogT_unit(cb, nt, pool=None):
            def go():
                tpsum = (pool or ps_sm).tile([P, P], bf16,
                                             tag="sm" if pool is None else "sc",
                                             name=f"t{cb}_{nt}")
                nc.tensor.transpose(
                    tpsum[:], og_sb[:, nt, cb * P : (cb + 1) * P], identity
                )
                copy_any(ogTs[cb][:, nt * P : (nt + 1) * P], tpsum[:])
            return go

        yv = y[:].rearrange("(nt p) c -> p nt c", p=P)

        def proj_unit(nt):
            def go():
                y_sb = ypool.tile([P, C], f32, tag="y", name=f"y{nt}")
                ppsum = ps_sc.tile([P, 1024], f32, tag="sc", name=f"pj{nt}")
                for halfc in range(2):
                    for cb in range(EG // P):
                        nc.tensor.matmul(
                            ppsum[:, halfc * 512 : halfc * 512 + EG],
                            ogTs[cb][:, nt * P : (nt + 1) * P],
                            wp_sb[:, cb, halfc * EG : (halfc + 1) * EG],
                            start=(cb == 0),
                            stop=(cb == EG // P - 1),
                        )
                    copy_any(
                        y_sb[:, halfc * EG : (halfc + 1) * EG],
                        ppsum[:, halfc * 512 : halfc * 512 + EG],
                    )
                    nc.sync.dma_start(
                        yv[:, nt, halfc * EG : (halfc + 1) * EG],
                        y_sb[:, halfc * EG : (halfc + 1) * EG],
                    )
            return go

        # ---- emission schedule ----
        # Phase i = (pair p=i//2, half hf=i%2). Pair p's scores need its
        # K block (3+p) over BOTH key halves, and Q block p for half hf —
        # emitted at least one phase ahead.
        qk_unit(3, 0)()
        qk_unit(3, 1)()
        qk_unit(0, 0)()
        pending = {
            0: [v_unit(m) for m in range(NT)] + [qk_unit(0, 1)],
            1: [qk_unit(4, 0), qk_unit(4, 1), qk_unit(1, 0)],
            2: [qk_unit(1, 1)],
            3: [qk_unit(5, 0), qk_unit(5, 1), qk_unit(2, 0)]
               + [ogT_unit(0, nt) for nt in range(NT)],
            4: [qk_unit(2, 1)],
            5: [ogT_unit(1, nt) for nt in range(NT)],
        }
        prev_pv = []
        for i in range(3 * NHALF):
            p, hf = i // NHALF, i % NHALF
            work = list(prev_pv) + pending.get(i, [])
            pts = emit_scores_pair(p, hf, work)
            prev_pv = pv_units(p, hf, pts)
        # tail: ogT2/proj for the first half (whose PV is already emitted)
        # interleave with the last pair's second-half PV, then the rest.
        ot2 = [ogT_unit(2, nt, pool=ps_sc) for nt in range(NT)]
        for nt in range(4):
            ot2[nt]()
        prev_pv[0]()
        prev_pv[1]()
        for nt in range(4, 8):
            ot2[nt]()
        for nt in range(4):
            proj_unit(nt)()
        prev_pv[2]()
        prev_pv[3]()
        for nt in range(4, 8):
            proj_unit(nt)()
        for nt in range(8, NT):
            ot2[nt]()
            proj_unit(nt)()

    nc.compile()
    return nc


_PROGRAM = None


def _get_program():
    global _PROGRAM
    if _PROGRAM is None:
        _PROGRAM = _build_program()
    return _PROGRAM


def _shard_inputs(x, Wqkv, Wproj):
    bf = ml_dtypes.bfloat16
    in_maps = []
    for core in range(NCORES):
        b, g = core // G, core % G
        xT = np.ascontiguousarray(x[b].T).astype(bf)
        wg = np.concatenate(
            [
                Wqkv[:, g * EG : (g + 1) * EG],
                Wqkv[:, C + g * EG : C + (g + 1) * EG],
                Wqkv[:, 2 * C + g * EG : 2 * C + (g + 1) * EG],
            ],
            axis=1,
        ).astype(bf)
        wp = np.ascontiguousarray(Wproj[g * EG : (g + 1) * EG, :]).astype(bf)
        in_maps.append({"xT": xT, "wqkv": wg, "wproj": wp})
    return in_maps


def _run(x, Wqkv, Wproj, bproj, trace=False):
    nc = _get_program()
    in_maps = _shard_inputs(x, Wqkv, Wproj)
    res = run_bass_kernel_spmd(nc, in_maps, list(range(NCORES)), trace=trace)
    out = np.empty((B, N, C), np.float32)
    for b in range(B):
        out[b] = res.results[b * G]["y"] + res.results[b * G + 1]["y"] + bproj
    return out, res


def kernel(x, Wqkv, Wproj, bproj):
    x = np.asarray(x, np.float32)
    Wqkv = np.asarray(Wqkv, np.float32)
    Wproj = np.asarray(Wproj, np.float32)
    bproj = np.asarray(bproj, np.float32)
    out, _ = _run(x, Wqkv, Wproj, bproj)
    return out

